# revision 26
# baseline (speedup 1.0000x reference)
"""MiniGPT forward on 8 Trainium2 NeuronCores.

Sharding: token-split data parallelism. Core c owns the token half
s = c%2 (rows [s*TLOC, (s+1)*TLOC)) of sequence p = c//2. All weight
GEMMs, layernorms and the tied head are per-token and run fully local;
the only cross-core dependency is attention keys/values, exchanged
per layer with AllGathers inside each core pair {2p, 2p+1}.

On-chip layout is feature-major: activations live as [128, n_tiles,
TLOC] SBUF tiles (feature on partitions, tokens on the free dim), so
every GEMM contraction dim is on partitions and weights stream in
pre-transposed (host-side) as [128, k_tile, out_features] bf16.

Key scheduling structure per layer:
  LN1 -> K-GEMM -> [AllGather K] -> V^T-GEMM (role-swapped matmul so V
  comes out token-major, no PE transposes) -> [AllGather V in 2 head
  halves] -> Q-GEMM -> attention (QK^T row-packed 2 heads/array,
  softmax denominator via an appended ones-column in V^T) -> proj ->
  LN2 -> fc1 -> fc2 (single pass, 8-deep PSUM).
The gathers are launched as soon as their producer GEMM drains, so
they hide entirely under the remaining GEMM work and the PE never
idles long enough to lose its HAM warm clock.

LayerNorm gain/bias are folded host-side into the following GEMM's
weights/biases (exact). The V-channel bias and lnf bias must be zero
(they are for this model); host_prep asserts this.
"""

import numpy as np
import ml_dtypes

P = 128

FULL_CFG = dict(B=4, T=1024, D=1024, H=16, HD=64, L=4, FF=4096, V=32000, NC=8,
                VCH=500, EPS=1e-5)


def _derived(cfg):
    d = dict(cfg)
    d["TLOC"] = cfg["B"] * cfg["T"] // cfg["NC"]     # tokens per core
    d["DT"] = cfg["D"] // P                          # d-model tiles
    d["QT"] = 3 * cfg["D"] // P                      # qkv output tiles
    d["FT"] = cfg["FF"] // P                         # ffn hidden tiles
    d["KTT"] = cfg["T"] // P                         # key tiles (full seq)
    d["MT"] = d["TLOC"] // P                         # token tiles per core
    d["NVC"] = cfg["V"] // cfg["VCH"]                # head vocab chunks
    d["HPT"] = P // cfg["HD"]                        # heads per 128-tile
    assert cfg["T"] % (2 * P) == 0 and d["TLOC"] * cfg["NC"] == cfg["B"] * cfg["T"]
    assert cfg["V"] % cfg["VCH"] == 0 and cfg["D"] % P == 0 and cfg["FF"] % P == 0
    assert d["TLOC"] <= 512, "single n-chunk design assumes TLOC <= 512"
    assert cfg["NC"] == 2 * cfg["B"], "one core pair per sequence"
    assert d["HPT"] == 2 and cfg["H"] % 2 == 0
    return d


def build_nc(cfg):
    import concourse.bass as bass
    import concourse.mybir as mybir
    import concourse.tile as tile
    from concourse import bacc

    f32 = mybir.dt.float32
    bf16 = mybir.dt.bfloat16
    AL = mybir.AluOpType
    AF = mybir.ActivationFunctionType

    c = _derived(cfg)
    B, T, D, H, HD, L, FF, V, NC = (cfg[k] for k in
                                    ("B", "T", "D", "H", "HD", "L", "FF", "V", "NC"))
    TLOC, DT, QT, FT, KTT, MT, NVC, HPT, VCH = (
        c[k] for k in ("TLOC", "DT", "QT", "FT", "KTT", "MT", "NVC", "HPT", "VCH"))
    EPS = cfg["EPS"]
    QSCALE = 1.0 / np.sqrt(HD)
    HH = H // 2                                      # heads per exchange half
    groups = [[2 * p, 2 * p + 1] for p in range(NC // 2)]

    nc = bacc.Bacc("TRN2", target_bir_lowering=False, debug=False, num_devices=NC)

    def din(name, shape, dt=f32):
        return nc.dram_tensor(name, list(shape), dt, kind="ExternalInput")

    # weights arrive pre-chunked in DMA consumption order so every weight
    # DMA is one dense 8KB-per-partition block (strided slices of the
    # naive layouts only get 1KB lines and ~1/3 of HBM bandwidth)
    h0T = din("h0T", [P, DT, TLOC])
    wqkvC = din("wqkvC", [L, 6, P, DT, 4 * P], bf16)  # K0 K1 V0 V1 Q0 Q1
    wprojC = din("wprojC", [L, 2, P, DT, 4 * P], bf16)
    wfc1C = din("wfc1C", [L, FT // 4, P, DT, 4 * P], bf16)
    wfc2T = din("wfc2T", [L, P, FT, D], bf16)
    qkv_b = din("qkv_b", [L, P, QT])                 # LN1 bias folded in
    proj_b = din("proj_b", [L, P, DT])
    fc1_b = din("fc1_b", [L, P, FT])                 # LN2 bias folded in
    fc2_b = din("fc2_b", [L, P, DT])
    masks = din("masks", [P, KTT, TLOC], bf16)
    embC = din("embC", [NVC, P, DT, VCH], bf16)      # lnf gain folded in
    logitsC = nc.dram_tensor("logitsC", [NVC, TLOC, VCH], bf16,
                             kind="ExternalOutput")

    with tile.TileContext(nc) as tc:
        import contextlib
        ctx = contextlib.ExitStack()
        with ctx:
            persist = ctx.enter_context(tc.tile_pool(name="persist", bufs=1))
            # PSUM: PA 4x1 bank + PB 2x2 banks = all 8 banks
            PA = ctx.enter_context(tc.tile_pool(name="PA", bufs=4, space="PSUM"))
            PB = ctx.enter_context(tc.tile_pool(name="PB", bufs=2, space="PSUM"))
            wpool = ctx.enter_context(tc.tile_pool(name="wpool", bufs=4))
            bigpool = ctx.enter_context(tc.tile_pool(name="bigpool", bufs=1))
            actpool = ctx.enter_context(tc.tile_pool(name="actpool", bufs=2))
            qpool = ctx.enter_context(tc.tile_pool(name="qpool", bufs=1))
            koutpool = ctx.enter_context(tc.tile_pool(name="koutpool", bufs=1))
            vtpool = ctx.enter_context(tc.tile_pool(name="vtpool", bufs=1))
            kpool = ctx.enter_context(tc.tile_pool(name="kpool", bufs=1))
            vspool = ctx.enter_context(tc.tile_pool(name="vspool", bufs=1))
            p_pool = ctx.enter_context(tc.tile_pool(name="p_pool", bufs=2))
            rowpool = ctx.enter_context(tc.tile_pool(name="rowpool", bufs=6))
            tmppool = ctx.enter_context(tc.tile_pool(name="tmppool", bufs=2))
            parpool = ctx.enter_context(tc.tile_pool(name="parpool", bufs=2))
            outpool = ctx.enter_context(tc.tile_pool(name="outpool", bufs=2))
            drampool = ctx.enter_context(tc.tile_pool(name="drampool", bufs=2,
                                                      space="DRAM"))

            zero_col = persist.tile([P, 1], f32)
            nc.vector.memset(zero_col[:], 0.0)
            nc.const_aps.aps[(f32, 0.0)] = zero_col[:]
            eps_col = persist.tile([P, 1], f32)
            nc.vector.memset(eps_col[:], EPS)
            nc.const_aps.aps[(f32, EPS)] = eps_col[:]

            invD_col = persist.tile([P, 1], bf16)
            nc.vector.memset(invD_col[:], 1.0 / D)
            ones_b = persist.tile([P, P], bf16)
            nc.vector.memset(ones_b[:], 1.0)

            masks_sb = persist.tile([P, KTT, TLOC], bf16)
            nc.scalar.dma_start(masks_sb[:], masks.ap())

            h_sb = persist.tile([P, DT, TLOC], f32)
            nc.sync.dma_start(h_sb[:], h0T.ap())
            # bf16 shadow of h, kept in sync at every residual write; LN
            # stats run off it so the stat matmuls go at bf16 rate
            h_bf = persist.tile([P, DT, TLOC], bf16)
            for t in range(DT):
                nc.scalar.copy(h_bf[:, t, :], h_sb[:, t, :])



            def layernorm(out_ap_fn, ntiles=DT):
                """(h - mean) * rsqrt(var+eps) -> bf16 tiles, affine folded
                into the consuming weights host-side."""
                sum_ps = PA.tile([1, TLOC], f32, tag="pa", name="ln_sum")
                sq_ps = PA.tile([1, TLOC], f32, tag="pa", name="ln_sq")
                for t in range(ntiles):
                    nc.tensor.matmul(sum_ps[:], invD_col[:, 0:1],
                                     h_bf[:, t, :],
                                     start=(t == 0), stop=(t == ntiles - 1))
                    hsq = tmppool.tile([P, TLOC], bf16, tag="hsq", name="hsq")
                    nc.vector.tensor_tensor(hsq[:], h_bf[:, t, :], h_bf[:, t, :],
                                            AL.mult)
                    nc.tensor.matmul(sq_ps[:], invD_col[:, 0:1],
                                     hsq[:],
                                     start=(t == 0), stop=(t == ntiles - 1))
                mean = rowpool.tile([1, TLOC], f32, tag="row", name="mean")
                nc.vector.tensor_copy(mean[:], sum_ps[:])
                m2 = rowpool.tile([1, TLOC], f32, tag="row", name="m2")
                nc.vector.tensor_tensor(m2[:], mean[:], mean[:], AL.mult)
                var = rowpool.tile([1, TLOC], f32, tag="row", name="var")
                nc.vector.tensor_tensor(var[:], sq_ps[:], m2[:], AL.subtract)
                std = rowpool.tile([1, TLOC], f32, tag="row", name="std")
                nc.scalar.activation(std[:], var[:], AF.Sqrt, bias=EPS)
                rstd = rowpool.tile([1, TLOC], f32, tag="row", name="rstd")
                nc.vector.reciprocal_approx_fast(rstd[:], std[:])
                mean_b = rowpool.tile([1, TLOC], bf16, tag="rowb", name="mean_b")
                nc.vector.tensor_copy(mean_b[:], mean[:])
                rstd_b = rowpool.tile([1, TLOC], bf16, tag="rowb", name="rstd_b")
                nc.vector.tensor_copy(rstd_b[:], rstd[:])
                mean_bc = PA.tile([P, TLOC], f32, tag="pa", name="mean_bc")
                nc.tensor.matmul(mean_bc[:], ones_b[0:1, :],
                                 mean_b[:], start=True, stop=True)
                rstd_bc = PA.tile([P, TLOC], f32, tag="pa", name="rstd_bc")
                nc.tensor.matmul(rstd_bc[:], ones_b[0:1, :],
                                 rstd_b[:], start=True, stop=True)
                for t in range(ntiles):
                    tmp = tmppool.tile([P, TLOC], bf16, tag="lntmp", name="lntmp")
                    nc.vector.tensor_sub(tmp[:], h_sb[:, t, :], mean_bc[:])
                    nc.vector.tensor_tensor(out_ap_fn(t), tmp[:], rstd_bc[:],
                                            AL.mult)

            def load_par(src, l, width):
                t = parpool.tile([P, width], f32, tag=f"par{width}", name="par")
                nc.sync.dma_start(t[:], src.ap()[l])
                return t

            for l in range(L):
                a_in = actpool.tile([P, DT, TLOC], bf16, tag="a_in", name="a_in1")
                layernorm(lambda t: a_in[:, t, :])

                qb = load_par(qkv_b, l, QT)
                wq_ap = wqkvC.ap()[l]  # [6, P, DT, 512]: K0 K1 V0 V1 Q0 Q1

                # ---- K and V^T GEMMs in interleaved head-halves, each
                # half exchanged as soon as its drains land, so the first
                # collective is in flight ~7us into the layer and pairs 0-3
                # only wait on the first K/V halves.  V^T: tokens on
                # partitions (stationary operand is the activation tile,
                # moving operand is the weight), so PV needs no transposes;
                # an appended ones column produces the softmax denominator
                # during PV.
                k_rem = []
                vt_rem = []
                for half in range(2):
                    kout = koutpool.tile([P, DT // 2, TLOC], bf16,
                                         tag=f"ko{half}", name=f"kout{half}")
                    wt = wpool.tile([P, DT, 4 * P], bf16, tag="w", name="wk")
                    nc.sync.dma_start(wt[:], wq_ap[half])
                    for m in range(4):
                        mt = half * 4 + m
                        ps = PA.tile([P, TLOC], f32, tag="pa", name="k_ps")
                        for k in range(DT):
                            nc.tensor.matmul(ps[:], wt[:, k, m * P:(m + 1) * P],
                                             a_in[:, k, :],
                                             start=(k == 0), stop=(k == DT - 1))
                        nc.vector.tensor_scalar_add(kout[:, m, :], ps[:],
                                                    qb[:, DT + mt:DT + mt + 1])
                    # staging DMAs ride the scalar queue: the sync queue is
                    # busy streaming weights and would delay the doorbell
                    k_in = drampool.tile([P, DT // 2, TLOC], bf16,
                                         tag=f"kin{half}", name="k_in")
                    nc.scalar.dma_start(k_in[:], kout[:])
                    k_out = drampool.tile([2, P, DT // 2, TLOC], bf16,
                                          tag=f"kout{half}", name="k_out")
                    nc.gpsimd.collective_compute(
                        "AllGather", AL.bypass, replica_groups=groups,
                        ins=[k_in[:].opt()], outs=[k_out[:].opt()])
                    ksb = kpool.tile([P, 2, DT // 2, TLOC], bf16,
                                     tag=f"ksb{half}", name=f"ksb{half}")
                    k_rem.append(ksb)
                    for sd in range(2):
                        nc.gpsimd.dma_start(ksb[:, sd], k_out[sd])

                    vt = vtpool.tile([P, MT, HH, HD + 1], bf16,
                                     tag=f"vt{half}", name=f"vt{half}")
                    nc.vector.memset(vt[:, :, :, HD:HD + 1], 1.0)
                    wv = wpool.tile([P, DT, 4 * P], bf16, tag="w", name="wv")
                    nc.sync.dma_start(wv[:], wq_ap[2 + half])
                    for m in range(MT):
                        ps = PA.tile([P, HH, HD], f32, tag="pa", name="vt_ps")
                        for k in range(DT):
                            nc.tensor.matmul(ps[:], a_in[:, k, m * P:(m + 1) * P],
                                             wv[:, k, :],
                                             start=(k == 0), stop=(k == DT - 1))
                        nc.vector.tensor_copy(vt[:, m, :, 0:HD], ps[:])
                    v_in = drampool.tile([P, MT, HH, HD + 1], bf16,
                                         tag=f"vin{half}", name="v_in")
                    nc.scalar.dma_start(v_in[:], vt[:])
                    v_out = drampool.tile([2, P, MT, HH, HD + 1], bf16,
                                          tag=f"vout{half}", name="v_out")
                    nc.gpsimd.collective_compute(
                        "AllGather", AL.bypass, replica_groups=groups,
                        ins=[v_in[:].opt()], outs=[v_out[:].opt()])
                    vsb = vspool.tile([P, 2, MT, HH, HD + 1], bf16,
                                      tag=f"vsb{half}", name=f"vsb{half}")
                    vt_rem.append(vsb)
                    for sd in range(2):
                        nc.gpsimd.dma_start(vsb[:, sd], v_out[sd])

                # ---- Q GEMM, interleaved with attention pairs: Q tile i
                # is the head-pair-i query block, so emit tile i+2 between
                # attention pairs.  The Q matmuls give the PE dense work
                # while the scalar engine (softmax exp) paces attention,
                # keeping the PE's HAM clock warm.
                q_sb = qpool.tile([P, DT, TLOC], bf16, name="q_sb")
                wt_q = {}

                def q_tile(mt):
                    ch = mt // 4
                    if mt % 4 == 0:
                        wt_q[ch] = wpool.tile([P, DT, 4 * P], bf16, tag="w",
                                              name="wq")
                        nc.sync.dma_start(wt_q[ch][:], wq_ap[4 + ch])
                    ps = PA.tile([P, TLOC], f32, tag="pa", name="q_ps")
                    for k in range(DT):
                        nc.tensor.matmul(
                            ps[:], wt_q[ch][:, k, (mt % 4) * P:(mt % 4 + 1) * P],
                            a_in[:, k, :], start=(k == 0), stop=(k == DT - 1))
                    nc.vector.tensor_scalar_add(q_sb[:, mt, :], ps[:],
                                                qb[:, mt:mt + 1])

                q_tile(0)
                q_tile(1)

                # ---- attention, head pairs; QK^T row-packed (head A in PE
                # rows 0-63, head B in rows 64-127, concurrent).  Scores and
                # PV alternate in half-key-range segments so the exp'd score
                # buffer is half-sized and PV overlaps the next score block.
                attout_sb = actpool.tile([P, DT, TLOC], bf16, tag="a_in",
                                         name="attout")
                for i in range(H // 2):
                    if i + 2 < DT:
                        q_tile(i + 2)
                    vsb = vt_rem[i // (HH // 2)]
                    ksb = k_rem[i // (HH // 2)]
                    ik = i % (HH // 2)
                    hhA, hhB = (2 * i) % HH, (2 * i + 1) % HH
                    aps = [PA.tile([P, TLOC], f32, tag="pa", name="av_ps")
                           for _ in range(2)]
                    NSEG = KTT // 2
                    for seg in range(NSEG):
                        pAB = p_pool.tile([P, 2, 2, TLOC], bf16,
                                          tag="P", name="pAB")
                        for q in range(2):
                            j = seg * 2 + q
                            sd, jj = j // (KTT // 2), j % (KTT // 2)
                            sp = PB.tile([P, 2, TLOC], f32, tag="pb",
                                         name="s_ps")
                            for a in range(2):
                                hb = a * HD
                                nc.tensor.matmul(
                                    sp[:, a, :],
                                    ksb[hb:hb + HD, sd, ik, jj * P:(jj + 1) * P],
                                    q_sb[hb:hb + HD, i, :],
                                    start=True, stop=True)
                            nc.scalar.activation(pAB[:, q, :, :], sp[:],
                                                 AF.Exp, scale=QSCALE)
                            for a in range(2):
                                nc.vector.tensor_tensor(
                                    pAB[:, q, a, :], pAB[:, q, a, :],
                                    masks_sb[:, j, :], AL.mult)
                        for a, hh in ((0, hhA), (1, hhB)):
                            for q in range(2):
                                j = seg * 2 + q
                                sd, jj = j // (KTT // 2), j % (KTT // 2)
                                nc.tensor.matmul(
                                    aps[a][0:HD + 1, :],
                                    vsb[:, sd, jj, hh, :],
                                    pAB[:, q, a, :],
                                    start=(j == 0), stop=(j == KTT - 1))
                    for a in range(2):
                        ap = aps[a]
                        attraw = tmppool.tile([HD, TLOC], bf16, tag="attraw",
                                              name="attraw")
                        nc.vector.tensor_copy(attraw[:], ap[0:HD, :])
                        den = rowpool.tile([1, TLOC], f32, tag="row", name="den")
                        nc.vector.tensor_copy(den[:], ap[HD:HD + 1, :])
                        rec = rowpool.tile([1, TLOC], f32, tag="row", name="rec")
                        nc.vector.reciprocal_approx_fast(rec[:], den[:])
                        rec_b = rowpool.tile([1, TLOC], bf16, tag="rowb",
                                             name="rec_b")
                        nc.vector.tensor_copy(rec_b[:], rec[:])
                        db = PA.tile([P, TLOC], f32, tag="pa", name="db_ps")
                        nc.tensor.matmul(db[0:HD, :], ones_b[0:1, 0:HD],
                                         rec_b[:], start=True, stop=True)
                        nc.vector.tensor_tensor(attout_sb[a * HD:(a + 1) * HD,
                                                          i, :],
                                                attraw[:], db[0:HD, :],
                                                AL.mult)

                # ---- proj GEMM + residual
                pb = load_par(proj_b, l, DT)
                for ch in range(2):
                    wt = wpool.tile([P, DT, 4 * P], bf16, tag="w", name="wproj")
                    nc.sync.dma_start(wt[:], wprojC.ap()[l][ch])
                    for m in range(4):
                        mt = ch * 4 + m
                        ps = PA.tile([P, TLOC], f32, tag="pa", name="proj_ps")
                        for k in range(DT):
                            nc.tensor.matmul(ps[:], wt[:, k, m * P:(m + 1) * P],
                                             attout_sb[:, k, :],
                                             start=(k == 0), stop=(k == DT - 1))
                        nc.vector.scalar_tensor_tensor(h_sb[:, mt, :], ps[:],
                                                       pb[:, mt:mt + 1],
                                                       h_sb[:, mt, :],
                                                       AL.add, AL.add)
                        nc.scalar.copy(h_bf[:, mt, :], h_sb[:, mt, :])

                # ---- mlp
                a2 = actpool.tile([P, DT, TLOC], bf16, tag="a_in", name="a_in2")
                layernorm(lambda t: a2[:, t, :])

                f1b = load_par(fc1_b, l, FT)
                mact = bigpool.tile([P, FT, TLOC], bf16, tag="big", name="mact")
                w1_ap = wfc1C.ap()[l]  # [8, P, DT, 512]
                for ch in range(FT // 4):
                    wt = wpool.tile([P, DT, 4 * P], bf16, tag="w", name="wfc1")
                    nc.sync.dma_start(wt[:], w1_ap[ch])
                    for m in range(4):
                        mt = ch * 4 + m
                        ps = PA.tile([P, TLOC], f32, tag="pa", name="fc1_ps")
                        for k in range(DT):
                            nc.tensor.matmul(ps[:], wt[:, k, m * P:(m + 1) * P],
                                             a2[:, k, :],
                                             start=(k == 0), stop=(k == DT - 1))
                        nc.scalar.activation(mact[:, mt, :], ps[:], AF.Gelu,
                                             bias=f1b[:, mt:mt + 1])

                # ---- fc2: single pass over mact, all 8 m-tiles' psums live
                f2b = load_par(fc2_b, l, DT)
                w2_ap = wfc2T.ap()[l]  # [P, FT, D]
                psA = [PA.tile([P, TLOC], f32, tag="pa", name=f"fc2a{m}")
                       for m in range(4)]
                psB = [PB.tile([P, 2, TLOC], f32, tag="pb", name=f"fc2b{m2}")
                       for m2 in range(2)]

                def fc2_ps(m):
                    return psA[m][:] if m < 4 else psB[(m - 4) // 2][:, (m - 4) % 2, :]

                for ch in range(FT // 4):
                    wt = wpool.tile([P, 4, D], bf16, tag="w", name="wfc2")
                    nc.sync.dma_start(wt[:], w2_ap[:, ch * 4:(ch + 1) * 4, :])
                    for m in range(DT):
                        for k in range(4):
                            kt = ch * 4 + k
                            nc.tensor.matmul(
                                fc2_ps(m), wt[:, k, m * P:(m + 1) * P],
                                mact[:, kt, :],
                                start=(kt == 0), stop=(kt == FT - 1))
                for m in range(DT):
                    nc.vector.scalar_tensor_tensor(
                        h_sb[:, m, :], fc2_ps(m), f2b[:, m:m + 1],
                        h_sb[:, m, :], AL.add, AL.add)
                    nc.scalar.copy(h_bf[:, m, :], h_sb[:, m, :])

            # ---- final layernorm + tied head
            af = actpool.tile([P, DT, TLOC], bf16, tag="a_in", name="a_f")
            layernorm(lambda t: af[:, t, :])

            for vc in range(NVC):
                ec = wpool.tile([P, DT, VCH], bf16, tag="w", name="ec")
                nc.sync.dma_start(ec[:], embC.ap()[vc])
                if vc % 2 == 0:
                    pss = [PA.tile([P, TLOC], f32, tag="pa", name="hd_ps")
                           for m in range(MT)]
                    pads = [p[:, 0:VCH] for p in pss]
                else:
                    pss = [PB.tile([P, 2, TLOC], f32, tag="pb", name="hd_ps2")
                           for m2 in range(MT // 2)]
                    pads = [pss[m // 2][:, m % 2, 0:VCH] for m in range(MT)]
                for m in range(MT):
                    for k in range(DT):
                        nc.tensor.matmul(pads[m], af[:, k, m * P:(m + 1) * P],
                                         ec[:, k, :],
                                         start=(k == 0), stop=(k == DT - 1))
                    ls = outpool.tile([P, VCH], bf16, tag="lout", name="ls")
                    if m % 2 == 0:
                        nc.vector.tensor_copy(ls[:], pads[m])
                    else:
                        nc.scalar.copy(ls[:], pads[m])
                    nc.sync.dma_start(
                        logitsC.ap()[vc][m * P:(m + 1) * P, :], ls[:])

    nc.compile()
    return nc


# ---------------------------------------------------------------------------
# host side
# ---------------------------------------------------------------------------

_CACHE = {}


def get_nc(cfg_key_and_cfg=None):
    cfg = FULL_CFG if cfg_key_and_cfg is None else cfg_key_and_cfg
    key = tuple(sorted(cfg.items()))
    if key not in _CACHE:
        _CACHE[key] = build_nc(cfg)
    return _CACHE[key]


def host_prep(inputs, cfg):
    """Build the per-core in_maps from full (unsharded) numpy inputs."""
    bf = ml_dtypes.bfloat16
    c = _derived(cfg)
    B, T, D, L, FF, V, NC = (cfg[k] for k in ("B", "T", "D", "L", "FF", "V", "NC"))
    TLOC, DT, QT, FT, KTT = (c[k] for k in ("TLOC", "DT", "QT", "FT", "KTT"))

    f = {k: np.asarray(v) for k, v in inputs.items()}
    x = f["x"].astype(np.int64)
    tok = f["tok_emb"].astype(np.float32)
    pos = f["pos_emb"].astype(np.float32)

    # Fold LN affine into the following GEMM (exact):
    #   W @ (z*g + b) + bw  ==  (W*g) @ z + (bw + W @ b)
    ln1_g = f["ln1_g"].astype(np.float32)
    ln1_b = f["ln1_b"].astype(np.float32)
    ln2_g = f["ln2_g"].astype(np.float32)
    ln2_b = f["ln2_b"].astype(np.float32)
    qkv_w = f["qkv_w"].astype(np.float32)
    fc1_w = f["fc1_w"].astype(np.float32)

    qkv_w_eff = qkv_w * ln1_g[:, None, :]
    qkv_b_eff = f["qkv_b"].astype(np.float32) + \
        np.einsum("lod,ld->lo", qkv_w, ln1_b)
    fc1_w_eff = fc1_w * ln2_g[:, None, :]
    fc1_b_eff = f["fc1_b"].astype(np.float32) + \
        np.einsum("lod,ld->lo", fc1_w, ln2_b)
    emb_eff = tok * f["lnf_g"].astype(np.float32)[None, :]

    assert np.abs(f["lnf_b"]).max() < 1e-6, "nonzero lnf bias unsupported"
    assert np.abs(qkv_b_eff[:, 2 * D:]).max() < 1e-6, \
        "nonzero V-channel bias unsupported"

    def wT_r(w, kdim, fdim):
        # [L, fdim, kdim] -> [L, 128, kdim/128, fdim] bf16
        wt = w.astype(np.float32).transpose(0, 2, 1)          # [L, kdim, fdim]
        wt = wt.reshape(L, kdim // P, P, fdim).transpose(0, 2, 1, 3)
        return np.ascontiguousarray(wt).astype(bf)

    def par_r(b, n):
        # [L, n*128] -> [L, 128, n]
        return np.ascontiguousarray(
            b.astype(np.float32).reshape(L, n, P).transpose(0, 2, 1))

    def chunked(wT, bases):
        # [L, P, kt, fdim] -> [L, nch, P, kt, 512] in DMA consumption order
        return np.ascontiguousarray(
            np.stack([wT[:, :, :, b:b + 4 * P] for b in bases], axis=1))

    wqkv_full = wT_r(qkv_w_eff, D, 3 * D)
    embT = np.ascontiguousarray(
        emb_eff.T.reshape(DT, P, V).transpose(1, 0, 2)).astype(bf)
    VCH = cfg["VCH"]
    NVC = V // VCH
    shared = {
        # chunk order: K0 K1 V0 V1 Q0 Q1
        "wqkvC": chunked(wqkv_full, [D, D + 512, 2 * D, 2 * D + 512, 0, 512]),
        "wprojC": chunked(wT_r(f["proj_w"], D, D), [0, 512]),
        "wfc1C": chunked(wT_r(fc1_w_eff, D, FF), list(range(0, FF, 512))),
        "wfc2T": wT_r(f["fc2_w"], FF, D),
        "qkv_b": par_r(qkv_b_eff, QT),
        "proj_b": par_r(f["proj_b"], DT),
        "fc1_b": par_r(fc1_b_eff, FT),
        "fc2_b": par_r(f["fc2_b"], DT),
        "embC": np.ascontiguousarray(
            embT.reshape(P, DT, NVC, VCH).transpose(2, 0, 1, 3)),
    }

    in_maps = []
    for core in range(NC):
        p, s = core // 2, core % 2
        h0 = tok[x[p]] + pos[:T]                              # [T, D]
        h0 = h0[s * TLOC:(s + 1) * TLOC]                      # [TLOC, D]
        h0T = np.ascontiguousarray(
            h0.T.reshape(DT, P, TLOC).transpose(1, 0, 2)).astype(np.float32)
        kt_g = (np.arange(KTT * P).reshape(KTT, P))           # [KTT, P]
        q_g = s * TLOC + np.arange(TLOC)
        m = (kt_g[:, :, None] <= q_g[None, None, :])          # [KTT, P, TLOC]
        m = np.ascontiguousarray(m.transpose(1, 0, 2)).astype(bf)
        in_maps.append(dict(shared, h0T=h0T, masks=m))
    return in_maps


def assemble(results, cfg):
    c = _derived(cfg)
    B, T, V = cfg["B"], cfg["T"], cfg["V"]
    TLOC = c["TLOC"]
    out = np.empty((B, T, V), np.float32)
    for core, r in enumerate(results):
        p, s = core // 2, core % 2
        lc = r["logitsC"].astype(np.float32)          # [NVC, TLOC, VCH]
        out[p, s * TLOC:(s + 1) * TLOC, :] = \
            lc.transpose(1, 0, 2).reshape(TLOC, V)
    return out


def run(inputs, cfg=None, **run_kwargs):
    from concourse.bass_utils import run_bass_kernel_spmd
    cfg = cfg or FULL_CFG
    nc = get_nc(cfg)
    in_maps = host_prep(inputs, cfg)
    res = run_bass_kernel_spmd(nc, in_maps, core_ids=list(range(cfg["NC"])),
                               **run_kwargs)
    return assemble(res.results, cfg), res


def kernel(**inputs) -> np.ndarray:
    out, _ = run(inputs, FULL_CFG)
    return out
